# revision 6
# baseline (speedup 1.0000x reference)
"""ConfGNN (APPNP propagation) Trainium2 Bass kernel, 8-core SPMD.

Strategy:
  - Nodes sharded across 8 cores (12500 each); edges partitioned by dst core.
  - Base model (x @ W_base -> softmax -> MLP) computed per-core on its node
    slice with PE matmuls (transposes via identity matmul).
  - APPNP: the normalized adjacency is separable: norm(e) =
    inv_out[src] * inv_in[dst].  We propagate g = h * inv_out; per hop each
    core gathers g[src] rows for its edges with dma_gather (int16 indices,
    windowed), scatter-adds them via one-hot matmuls into per-(window,range)
    PSUM tiles (contiguous accumulation groups on rotating PSUM banks --
    interleaved accumulation regions within a bank corrupt results), reduces
    windows into an SBUF fp32 aggregator, then blends
    h' = (1-a)*inv_in*agg + a*h0 and allgathers the updated g table.
  - All per-edge index/one-hot data is precomputed on the host and passed as
    extra DRAM inputs; the device program is identical on all 8 cores.
"""

import numpy as np

import concourse.bass as bass
import concourse.bacc as bacc
import concourse.mybir as mybir
import concourse.tile as tile
from concourse.masks import make_identity
from concourse.bass_utils import run_bass_kernel_spmd

N_CORES = 8
ALPHA = 0.1
K_HOPS = 10
F = 40          # feature/class dim
C_IN = 500      # input dim
WS = 25088      # gather window (positions per window), < int16 max
TROW = 128      # table row width (bf16 elems) = 256 B gather element
GCHUNK = 1024   # indices per dma_gather (hardware cap)
P = 128

_CACHE = {}


# ----------------------------------------------------------------------------
# Host preprocessing
# ----------------------------------------------------------------------------

def _prep(x, edge_index, W_base, b_base, W1, b1, W2, b2):
    N = x.shape[0]
    E = edge_index.shape[1]
    n_c = N // N_CORES                      # nodes per core
    R = (n_c + P - 1) // P                  # 128-ranges per core
    S_c = R * P                             # padded slots per core
    pos_total = N_CORES * S_c
    NW = (pos_total + WS - 1) // WS         # number of gather windows

    src = np.asarray(edge_index[0], dtype=np.int64)
    dst = np.asarray(edge_index[1], dtype=np.int64)

    deg_out = np.bincount(src, minlength=N).astype(np.float64)
    deg_in = np.bincount(dst, minlength=N).astype(np.float64)
    inv_out = np.where(deg_out > 0, 1.0 / np.sqrt(np.maximum(deg_out, 1.0)), 0.0)
    inv_in = np.where(deg_in > 0, 1.0 / np.sqrt(np.maximum(deg_in, 1.0)), 0.0)

    # per-edge placement
    c = dst // n_c
    slot = dst - c * n_c
    r = slot // P
    col = slot % P
    pos_src = (src // n_c) * S_c + (src % n_c)
    w = pos_src // WS
    iw = pos_src - w * WS

    # group edges by (core, window, range)
    key = (c * NW + w) * R + r
    order = np.argsort(key, kind="stable")
    counts = np.bincount(key, minlength=N_CORES * NW * R).reshape(N_CORES, NW, R)

    # shared sub-chunk plan: K[w][r] 128-edge sub-chunks, maxed over cores
    Kwr = (counts + P - 1) // P
    Kwr = Kwr.max(axis=0)                   # [NW, R]
    Kwr[0] = np.maximum(Kwr[0], 1)          # window 0 always opens each range

    M_w = Kwr.sum(axis=1)                   # sub-chunks per window
    n_chunks_w = (M_w * P + GCHUNK - 1) // GCHUNK
    TOT_w = n_chunks_w * GCHUNK             # padded edge-stream len per window
    M_total = int(M_w.sum())

    # stream offsets of each (w, r) group within its window stream
    grp_off = np.zeros((NW, R), dtype=np.int64)
    for wi in range(NW):
        grp_off[wi] = np.concatenate([[0], np.cumsum(Kwr[wi] * P)[:-1]])

    # per-core padded streams
    idx_cols = int((TOT_w // 16).sum())
    idx_all = np.zeros((N_CORES, P, idx_cols), dtype=np.int16)
    slotcol = np.full((N_CORES, P, M_total), -1, dtype=np.float32)

    sorted_key = key[order]
    sorted_iw = iw[order]
    sorted_col = col[order]
    # rank within group
    grp_start = np.searchsorted(sorted_key, np.arange(N_CORES * NW * R))
    rank = np.arange(E) - grp_start[sorted_key]
    ck = sorted_key // (NW * R)
    wk = (sorted_key // R) % NW
    rk = sorted_key % R

    win_col_off = np.concatenate([[0], np.cumsum(TOT_w // 16)[:-1]])
    m_off = np.concatenate([[0], np.cumsum(M_w)[:-1]])  # sub-chunk idx offset/win

    for core in range(N_CORES):
        sel = ck == core
        wi_e = wk[sel]
        ri_e = rk[sel]
        rank_e = rank[sel]
        iw_e = sorted_iw[sel]
        col_e = sorted_col[sel]
        # position within window stream
        spos = grp_off[wi_e, ri_e] + rank_e
        for wi in range(NW):
            ws_sel = wi_e == wi
            sp = spos[ws_sel]
            stream = np.zeros(int(TOT_w[wi]), dtype=np.int16)
            stream[sp] = iw_e[ws_sel].astype(np.int16)
            wrapped = stream.reshape(-1, 16).T          # [16, TOT_w/16]
            idx_all[core, :, win_col_off[wi]:win_col_off[wi] + wrapped.shape[1]] = \
                np.tile(wrapped, (8, 1))
            # slot columns for real sub-chunks of this window
            cstream = np.full(int(M_w[wi]) * P, -1, dtype=np.float32)
            cstream[sp] = col_e[ws_sel]
            slotcol[core, :, m_off[wi]:m_off[wi] + M_w[wi]] = \
                cstream.reshape(-1, P).T

    # matmul plan (shared across cores): per (w, r, k)
    # start/stop delimit a contiguous accumulation group per (w, r); at stop
    # the group result is copied (w==0) or added (w>0) into the SBUF agg.
    plan = []  # (m_index, r, start, stop, first_window)
    m = 0
    for wi in range(NW):
        for ri in range(R):
            kk = int(Kwr[wi][ri])
            for k in range(kk):
                plan.append((m, int(ri), k == 0, k == kk - 1, wi == 0))
                m += 1
    assert m == M_total

    # compact per-(partition, range) scalar tables (value for slot r*128+p)
    def slot_compact(vec):
        t = np.zeros((N_CORES, P, R), dtype=np.float32)
        v = vec.reshape(N_CORES, n_c)
        for core in range(N_CORES):
            vv = np.zeros(S_c, dtype=np.float64)
            vv[:n_c] = v[core]
            t[core] = vv.reshape(R, P).T
        return t

    A40c = slot_compact((1.0 - ALPHA) * inv_in * inv_out)
    Apc = slot_compact((1.0 - ALPHA) * inv_in)
    IOc = slot_compact(inv_out)

    # padded x slices
    xp = np.zeros((N_CORES, S_c, C_IN), dtype=np.float32)
    xr = np.asarray(x, dtype=np.float32).reshape(N_CORES, n_c, C_IN)
    xp[:, :n_c, :] = xr

    Wb = np.ascontiguousarray(
        np.asarray(W_base, np.float32).reshape(4, 125, F).transpose(1, 0, 2)
    ).reshape(125, 4 * F)

    meta = dict(
        N=N, E=E, n_c=n_c, R=R, S_c=S_c, pos_total=pos_total, NW=NW,
        M_total=M_total,
        n_chunks_w=tuple(int(v) for v in n_chunks_w),
        TOT_w=tuple(int(v) for v in TOT_w),
        M_w=tuple(int(v) for v in M_w),
        idx_cols=idx_cols, plan=tuple(plan),
        win_col_off=tuple(int(v) for v in win_col_off),
    )
    per_core = []
    for core in range(N_CORES):
        per_core.append({
            "xp": xp[core],
            "idx": idx_all[core],
            "slotcol": slotcol[core],
            "A40c": A40c[core], "Apc": Apc[core], "IOc": IOc[core],
            "Wb": Wb,
            "b_base": np.tile(np.asarray(b_base, np.float32).reshape(1, F), (128, 1)),
            "W1": np.asarray(W1, np.float32),
            "b1": np.tile(np.asarray(b1, np.float32).reshape(1, 64), (128, 1)),
            "W2": np.asarray(W2, np.float32),
            "b2": np.tile(np.asarray(b2, np.float32).reshape(1, F), (128, 1)),
        })
    return meta, per_core


# ----------------------------------------------------------------------------
# Bass program
# ----------------------------------------------------------------------------

def _build(meta, hops=K_HOPS):
    n_c, R, S_c = meta["n_c"], meta["R"], meta["S_c"]
    NW, M_total = meta["NW"], meta["M_total"]
    pos_total = meta["pos_total"]
    n_chunks_w = meta["n_chunks_w"]
    idx_cols = meta["idx_cols"]
    win_col_off = meta["win_col_off"]
    plan = meta["plan"]
    dt = mybir.dt

    nc = bacc.Bacc("TRN2", target_bir_lowering=False, debug=False,
                   num_devices=N_CORES, num_swdge_queues=4)

    # I/O
    xp = nc.dram_tensor("xp", [S_c, C_IN], dt.float32, kind="ExternalInput")
    idx_in = nc.dram_tensor("idx", [P, idx_cols], dt.int16, kind="ExternalInput")
    slotcol_in = nc.dram_tensor("slotcol", [P, M_total], dt.float32,
                                kind="ExternalInput")
    A40c_in = nc.dram_tensor("A40c", [P, R], dt.float32, kind="ExternalInput")
    Apc_in = nc.dram_tensor("Apc", [P, R], dt.float32, kind="ExternalInput")
    IOc_in = nc.dram_tensor("IOc", [P, R], dt.float32, kind="ExternalInput")
    Wb_in = nc.dram_tensor("Wb", [125, 4 * F], dt.float32, kind="ExternalInput")
    bb_in = nc.dram_tensor("b_base", [P, F], dt.float32, kind="ExternalInput")
    W1_in = nc.dram_tensor("W1", [F, 64], dt.float32, kind="ExternalInput")
    b1_in = nc.dram_tensor("b1", [P, 64], dt.float32, kind="ExternalInput")
    W2_in = nc.dram_tensor("W2", [64, F], dt.float32, kind="ExternalInput")
    b2_in = nc.dram_tensor("b2", [P, F], dt.float32, kind="ExternalInput")

    adjust_out = nc.dram_tensor("adjust", [n_c, F], dt.float32,
                                kind="ExternalOutput")
    scores_out = nc.dram_tensor("scores", [n_c, F], dt.float32,
                                kind="ExternalOutput")

    gs_dram = nc.dram_tensor("gs", [S_c, TROW], dt.bfloat16, kind="Internal")
    table = nc.dram_tensor("gtable", [pos_total, TROW], dt.bfloat16,
                           kind="Internal")

    with tile.TileContext(nc) as tc:
        with (tc.tile_pool(name="const", bufs=1) as constp,
              tc.tile_pool(name="resident", bufs=1) as resp):
            ident = constp.tile([P, P], dt.float32)
            make_identity(nc, ident[:])
            iota_i = constp.tile([P, P], dt.int32)
            nc.gpsimd.iota(iota_i[:], pattern=[[1, P]], base=0,
                           channel_multiplier=0)
            iota_f = constp.tile([P, P], dt.float32)
            nc.vector.tensor_copy(out=iota_f[:], in_=iota_i[:])

            idx_t = resp.tile([P, idx_cols], dt.int16)
            nc.sync.dma_start(out=idx_t[:], in_=idx_in[:])
            slotcol_t = resp.tile([P, M_total], dt.float32)
            nc.sync.dma_start(out=slotcol_t[:], in_=slotcol_in[:])
            A40c_t = resp.tile([P, R], dt.float32)
            nc.sync.dma_start(out=A40c_t[:], in_=A40c_in[:])
            Apc_t = resp.tile([P, R], dt.float32)
            nc.sync.dma_start(out=Apc_t[:], in_=Apc_in[:])
            IOc_t = resp.tile([P, R], dt.float32)
            nc.sync.dma_start(out=IOc_t[:], in_=IOc_in[:])

            h0_sb = resp.tile([P, R * F], dt.float32)      # [p, r*F]
            ah0_sb = resp.tile([P, R * F], dt.float32)     # alpha * h0
            bh0_sb = resp.tile([P, R * F], dt.float32)     # alpha * inv_out * h0
            g_slice = resp.tile([P, R * TROW], dt.bfloat16)
            nc.vector.memset(g_slice[:], 0.0)

            # ------------------- phase 1: base model + MLP -------------------
            with (tc.tile_pool(name="p1", bufs=3) as p1,
                  tc.tile_pool(name="p1w", bufs=1) as p1w,
                  tc.tile_pool(name="p1ps", bufs=1, space="PSUM") as p1ps):
                Wb_sb = p1w.tile([125, 4 * F], dt.float32)
                nc.sync.dma_start(out=Wb_sb[:], in_=Wb_in[:])
                W1_sb = p1w.tile([F, 64], dt.float32)
                nc.sync.dma_start(out=W1_sb[:], in_=W1_in[:])
                W2_sb = p1w.tile([64, F], dt.float32)
                nc.sync.dma_start(out=W2_sb[:], in_=W2_in[:])
                bb_sb = p1w.tile([P, F], dt.float32)
                nc.sync.dma_start(out=bb_sb[:], in_=bb_in[:])
                b1_sb = p1w.tile([P, 64], dt.float32)
                nc.sync.dma_start(out=b1_sb[:], in_=b1_in[:])
                b2_sb = p1w.tile([P, F], dt.float32)
                nc.sync.dma_start(out=b2_sb[:], in_=b2_in[:])

                for t in range(R):
                    x_t = p1.tile([P, C_IN], dt.float32, tag="xt")
                    nc.sync.dma_start(out=x_t[:], in_=xp[t * P:(t + 1) * P, :])
                    sc_ps = p1ps.tile([P, F], dt.float32, space="PSUM",
                                      tag="scps")
                    for ch in range(4):
                        xT_ps = p1ps.tile([125, P], dt.float32, space="PSUM",
                                          tag="xtps")
                        nc.tensor.transpose(
                            out=xT_ps[:],
                            in_=x_t[:, ch * 125:(ch + 1) * 125],
                            identity=ident[:])
                        xT_sb = p1.tile([125, P], dt.float32, tag="xtsb")
                        nc.vector.tensor_copy(out=xT_sb[:], in_=xT_ps[:])
                        nc.tensor.matmul(
                            out=sc_ps[:], lhsT=xT_sb[:],
                            rhs=Wb_sb[:, ch * F:(ch + 1) * F],
                            start=(ch == 0), stop=(ch == 3))
                    sc_sb = p1.tile([P, F], dt.float32, tag="scsb")
                    nc.vector.tensor_tensor(
                        out=sc_sb[:], in0=sc_ps[:],
                        in1=bb_sb[:],
                        op=mybir.AluOpType.add)
                    rows = min(P, n_c - t * P)
                    if rows > 0:
                        nc.sync.dma_start(
                            out=scores_out[t * P:t * P + rows, :],
                            in_=sc_sb[:rows, :])
                    # softmax
                    mx = p1.tile([P, 1], dt.float32, tag="mx")
                    nc.vector.tensor_reduce(
                        out=mx[:], in_=sc_sb[:], axis=mybir.AxisListType.X,
                        op=mybir.AluOpType.max, negate=True)
                    ex = p1.tile([P, F], dt.float32, tag="ex")
                    nc.scalar.activation(
                        out=ex[:], in_=sc_sb[:],
                        func=mybir.ActivationFunctionType.Exp,
                        bias=mx[:], scale=1.0)
                    sm_sum = p1.tile([P, 1], dt.float32, tag="sms")
                    nc.vector.tensor_reduce(
                        out=sm_sum[:], in_=ex[:], axis=mybir.AxisListType.X,
                        op=mybir.AluOpType.add)
                    rinv = p1.tile([P, 1], dt.float32, tag="rinv")
                    nc.vector.reciprocal(out=rinv[:], in_=sm_sum[:])
                    sm = p1.tile([P, F], dt.float32, tag="sm")
                    nc.vector.tensor_scalar_mul(sm[:], ex[:], rinv[:])
                    # MLP
                    oT_ps = p1ps.tile([F, P], dt.float32, space="PSUM",
                                      tag="otps")
                    nc.tensor.transpose(out=oT_ps[:], in_=sm[:],
                                        identity=ident[:])
                    oT_sb = p1.tile([F, P], dt.float32, tag="otsb")
                    nc.vector.tensor_copy(out=oT_sb[:], in_=oT_ps[:])
                    h1_ps = p1ps.tile([P, 64], dt.float32, space="PSUM",
                                      tag="h1ps")
                    nc.tensor.matmul(out=h1_ps[:], lhsT=oT_sb[:], rhs=W1_sb[:],
                                     start=True, stop=True)
                    h1b = p1.tile([P, 64], dt.float32, tag="h1b")
                    nc.vector.tensor_tensor(
                        out=h1b[:], in0=h1_ps[:],
                        in1=b1_sb[:],
                        op=mybir.AluOpType.add)
                    h1r = p1.tile([P, 64], dt.float32, tag="h1r")
                    nc.vector.tensor_scalar_max(h1r[:], h1b[:], 0.0)
                    h1T_ps = p1ps.tile([64, P], dt.float32, space="PSUM",
                                       tag="h1tps")
                    nc.tensor.transpose(out=h1T_ps[:], in_=h1r[:],
                                        identity=ident[:])
                    h1T_sb = p1.tile([64, P], dt.float32, tag="h1tsb")
                    nc.vector.tensor_copy(out=h1T_sb[:], in_=h1T_ps[:])
                    h_ps = p1ps.tile([P, F], dt.float32, space="PSUM",
                                     tag="hps")
                    nc.tensor.matmul(out=h_ps[:], lhsT=h1T_sb[:], rhs=W2_sb[:],
                                     start=True, stop=True)
                    nc.vector.tensor_tensor(
                        out=h0_sb[:, t * F:(t + 1) * F], in0=h_ps[:],
                        in1=b2_sb[:],
                        op=mybir.AluOpType.add)
                    # g0 = inv_out * h0
                    nc.vector.tensor_scalar(
                        out=g_slice[:, t * TROW:t * TROW + F],
                        in0=h0_sb[:, t * F:(t + 1) * F],
                        scalar1=IOc_t[:, t:t + 1], scalar2=None,
                        op0=mybir.AluOpType.mult)
                    # bh0 = alpha * inv_out * h0 ; ah0 = alpha * h0
                    nc.vector.tensor_scalar(
                        out=bh0_sb[:, t * F:(t + 1) * F],
                        in0=g_slice[:, t * TROW:t * TROW + F],
                        scalar1=ALPHA, scalar2=None,
                        op0=mybir.AluOpType.mult)
                    nc.vector.tensor_scalar(
                        out=ah0_sb[:, t * F:(t + 1) * F],
                        in0=h0_sb[:, t * F:(t + 1) * F],
                        scalar1=ALPHA, scalar2=None,
                        op0=mybir.AluOpType.mult)

            def push_table():
                nc.sync.dma_start(
                    out=gs_dram[:].rearrange("(t p) e -> p t e", p=P),
                    in_=g_slice[:].rearrange("p (t e) -> p t e", e=TROW))
                nc.gpsimd.collective_compute(
                    "AllGather", mybir.AluOpType.bypass,
                    replica_groups=[list(range(N_CORES))],
                    ins=[gs_dram[:]],
                    outs=[table[:]],
                )

            push_table()

            # ---------------------------- hops ----------------------------
            with (tc.tile_pool(name="msg", bufs=12) as msgp,
                  tc.tile_pool(name="oh", bufs=8) as ohp,
                  tc.tile_pool(name="bl", bufs=4) as blp,
                  tc.tile_pool(name="agps", bufs=8, space="PSUM") as agps):
                agg_sb = resp.tile([P, R * F], dt.float32)
                adj_sb = resp.tile([P, R * F], dt.float32)

                for hop in range(hops):
                    last = hop == hops - 1
                    pl = 0
                    qn = 0
                    cur_ps = None
                    for wi in range(NW):
                        base = wi * WS
                        wrows = min(WS, pos_total - base)
                        sub_in_w = 0
                        for j in range(n_chunks_w[wi]):
                            msg = msgp.tile([P, GCHUNK // P, TROW],
                                            dt.bfloat16, tag="msg")
                            ccol = win_col_off[wi] + j * (GCHUNK // 16)
                            nc.gpsimd.dma_gather(
                                out_ap=msg[:],
                                in_ap=table[base:base + wrows, :],
                                idxs_ap=idx_t[:, ccol:ccol + GCHUNK // 16],
                                num_idxs=GCHUNK, num_idxs_reg=GCHUNK,
                                elem_size=TROW, single_packet=True,
                                queue_num=qn % 4)
                            qn += 1
                            for jj in range(GCHUNK // P):
                                if sub_in_w >= meta["M_w"][wi]:
                                    break  # padded tail of this window
                                mm, ri, st, sp, firstw = plan[pl]
                                oh = ohp.tile([P, P], dt.bfloat16, tag="oh")
                                nc.vector.tensor_scalar(
                                    out=oh[:], in0=iota_f[:],
                                    scalar1=slotcol_t[:, mm:mm + 1],
                                    scalar2=None,
                                    op0=mybir.AluOpType.is_equal)
                                if st:
                                    cur_ps = agps.tile([P, F], dt.float32,
                                                       space="PSUM", tag="ag")
                                nc.tensor.matmul(
                                    out=cur_ps[:], lhsT=oh[:],
                                    rhs=msg[:, jj, 0:F],
                                    start=st, stop=sp,
                                    skip_group_check=True)
                                if sp:
                                    so = ri * F
                                    if firstw:
                                        nc.vector.tensor_copy(
                                            out=agg_sb[:, so:so + F],
                                            in_=cur_ps[:])
                                    else:
                                        nc.vector.tensor_tensor(
                                            out=agg_sb[:, so:so + F],
                                            in0=agg_sb[:, so:so + F],
                                            in1=cur_ps[:],
                                            op=mybir.AluOpType.add)
                                pl += 1
                                sub_in_w += 1
                    # blend
                    Ac = Apc_t if last else A40c_t
                    add_sb = ah0_sb if last else bh0_sb
                    for ri in range(R):
                        t1 = blp.tile([P, F], dt.float32, tag="t1")
                        nc.vector.tensor_scalar(
                            out=t1[:], in0=agg_sb[:, ri * F:(ri + 1) * F],
                            scalar1=Ac[:, ri:ri + 1], scalar2=None,
                            op0=mybir.AluOpType.mult)
                        if last:
                            nc.vector.tensor_tensor(
                                out=adj_sb[:, ri * F:(ri + 1) * F],
                                in0=t1[:], in1=add_sb[:, ri * F:(ri + 1) * F],
                                op=mybir.AluOpType.add)
                        else:
                            nc.vector.tensor_tensor(
                                out=g_slice[:, ri * TROW:ri * TROW + F],
                                in0=t1[:], in1=add_sb[:, ri * F:(ri + 1) * F],
                                op=mybir.AluOpType.add)
                    if not last:
                        push_table()

                # write adjust output (original order = slot order, minus pad)
                full_t = n_c // P
                nc.sync.dma_start(
                    out=adjust_out[0:full_t * P, :].rearrange(
                        "(t p) f -> p t f", p=P),
                    in_=adj_sb[:].rearrange(
                        "p (t f) -> p t f", f=F)[:, 0:full_t, :])
                rem = n_c - full_t * P
                if rem > 0:
                    nc.sync.dma_start(
                        out=adjust_out[full_t * P:, :],
                        in_=adj_sb[:rem, full_t * F:(full_t + 1) * F])

    nc.compile()
    return nc


# ----------------------------------------------------------------------------
# entry point
# ----------------------------------------------------------------------------

def _get_built(x, edge_index, W_base, b_base, W1, b1, W2, b2, hops=K_HOPS):
    meta, per_core = _prep(x, edge_index, W_base, b_base, W1, b1, W2, b2)
    sig = (meta["N"], meta["E"], meta["M_total"], meta["idx_cols"],
           meta["n_chunks_w"], hops)
    if sig not in _CACHE:
        _CACHE[sig] = _build(meta, hops=hops)
    return _CACHE[sig], meta, per_core


def kernel(x, edge_index, W_base, b_base, W1, b1, W2, b2):
    nc, meta, per_core = _get_built(
        x, edge_index, W_base, b_base, W1, b1, W2, b2)
    in_maps = [{k: np.ascontiguousarray(v) for k, v in pc.items()}
               for pc in per_core]
    res = run_bass_kernel_spmd(nc, in_maps, core_ids=list(range(N_CORES)))
    n_c = meta["n_c"]
    adjust = np.concatenate(
        [res.results[c]["adjust"] for c in range(N_CORES)], axis=0)
    scores = np.concatenate(
        [res.results[c]["scores"] for c in range(N_CORES)], axis=0)
    return adjust, scores
